# revision 1
# baseline (speedup 1.0000x reference)
import numpy as np

# nn_Attention: B=256, N=65, DIM=1024, HEADS=16, DH=64 across 8 cores (32 batches/core)
B, N, DIM, HEADS, DH = 256, 65, 1024, 16, 64
NCORES = 8
BPC = B // NCORES            # 32 batches per core
TOK = BPC * N                # 2080 tokens per core
BB = 4                       # batches per block
NBLK = BPC // BB             # 8 blocks
TB = BB * N                  # 260 tokens per block
BN_EPS = 1e-5


def _build(nc_mod, mybir, bass):
    f32 = mybir.dt.float32
    f32r = mybir.dt.float32r
    Alu = mybir.AluOpType
    Act = mybir.ActivationFunctionType
    from concourse.tile import TileContext

    nc = nc_mod
    xt = nc.declare_dram_parameter("xt", [DIM, TOK], f32r, isOutput=False)
    wqkvt = nc.declare_dram_parameter("wqkvt", [DIM, 3 * DIM], f32r, isOutput=False)
    woutt = nc.declare_dram_parameter("woutt", [DIM, DIM], f32r, isOutput=False)
    wconvt = nc.declare_dram_parameter("wconvt", [9, N, 68], f32r, isOutput=False)
    st = nc.declare_dram_parameter("st", [N, 2], f32, isOutput=False)
    bqkvc = nc.declare_dram_parameter("bqkvc", [128, 24], f32, isOutput=False)
    ident = nc.declare_dram_parameter("ident", [128, 128], f32, isOutput=False)
    vzpad = nc.declare_dram_parameter("vzpad", [N, 32], f32r, isOutput=False)
    out = nc.declare_dram_parameter("out", [TOK, DIM], f32, isOutput=True)

    from contextlib import ExitStack
    with TileContext(nc) as tc:
        with ExitStack() as es:
            P = lambda *a, **k: es.enter_context(tc.tile_pool(*a, **k))
            cp = P(name="consts", bufs=1)
            xtp = P(name="xtp", bufs=1)
            qtp = P(name="qtp", bufs=1)
            ktp = P(name="ktp", bufs=1)
            vxp = P(name="vxp", bufs=2)
            vtp = P(name="vtp", bufs=1)
            vpp = P(name="vpp", bufs=2)
            resp = P(name="resp", bufs=2)
            bnp = P(name="bnp", bufs=1)
            rtp = P(name="rtp", bufs=2)
            osbp = P(name="osb", bufs=2)
            expp = P(name="exps", bufs=4)
            recp = P(name="recips", bufs=4)
            pa = P(name="pa", bufs=1, space="PSUM")
            p512 = P(name="p512", bufs=2, space="PSUM")
            ps = P(name="ps", bufs=3, space="PSUM")
            pcv = P(name="pconv", bufs=2, space="PSUM")
            # ---- resident constants ----
            wqkv_sb = cp.tile([128, 8 * 3 * DIM], f32r, tag="wqkv")
            nc.sync.dma_start(
                out=wqkv_sb[:].rearrange("p (a n) -> p a n", a=8),
                in_=wqkvt[:].rearrange("(a p) n -> p a n", p=128),
            )
            wqkv = wqkv_sb[:].rearrange("p (a n) -> p a n", a=8)

            wout_sb = cp.tile([128, 8 * DIM], f32r, tag="wout")
            nc.sync.dma_start(
                out=wout_sb[:].rearrange("p (a n) -> p a n", a=8),
                in_=woutt[:].rearrange("(a p) n -> p a n", p=128),
            )
            wout = wout_sb[:].rearrange("p (a n) -> p a n", a=8)

            wconv_sb = cp.tile([N, 9 * 68], f32r, tag="wconv")
            nc.sync.dma_start(
                out=wconv_sb[:].rearrange("i (t o) -> i t o", t=9),
                in_=wconvt[:].rearrange("t i o -> i t o"),
            )
            wconv = wconv_sb[:].rearrange("i (t o) -> i t o", t=9)
            vz_sb = cp.tile([N, 32], f32r, tag="vz")
            nc.sync.dma_start(out=vz_sb[:], in_=vzpad[:])

            st_sb = cp.tile([N, 2], f32, tag="st")
            nc.sync.dma_start(out=st_sb[:], in_=st[:])
            bq_sb = cp.tile([128, 24], f32, tag="bq")
            nc.sync.dma_start(out=bq_sb[:], in_=bqkvc[:])
            id_sb = cp.tile([128, 128], f32, tag="id")
            nc.sync.dma_start(out=id_sb[:], in_=ident[:])

            for blk in range(NBLK):
                t0 = blk * TB
                xt_sb = xtp.tile([128, 8 * TB], f32r, tag="xt")
                xtv = xt_sb[:].rearrange("p (a n) -> p a n", a=8)
                nc.sync.dma_start(
                    out=xtv,
                    in_=xt[:].rearrange("(a p) n -> p a n", p=128)[:, :, t0:t0 + TB],
                )

                # ---- Q^T, K^T, V^T projections: [feat 128-tile, TB] ----
                qt_sb = qtp.tile([128, 8 * TB], f32, tag="qt")
                qtv = qt_sb[:].rearrange("p (a n) -> p a n", a=8)
                kt_sb = ktp.tile([128, 8 * TB], f32, tag="kt")
                ktv = kt_sb[:].rearrange("p (a n) -> p a n", a=8)
                vt_sb = vtp.tile([128, 8 * TB], f32, tag="vt")
                vtv = vt_sb[:].rearrange("p (a n) -> p a n", a=8)
                for dst, coff, boff in ((qtv, 0, 0), (ktv, DIM, 8), (vtv, 2 * DIM, 16)):
                    for m in range(8):
                        pqk = pa.tile([128, TB], f32, tag="pA")
                        for ki in range(8):
                            nc.tensor.matmul(
                                pqk[:],
                                wqkv[:, ki, coff + m * 128:coff + (m + 1) * 128],
                                xtv[:, ki, :],
                                start=(ki == 0), stop=(ki == 7),
                            )
                        nc.vector.tensor_scalar_add(
                            dst[:, m, :], pqk[:], bq_sb[:, boff + m:boff + m + 1]
                        )

                for bi in range(BB):
                    toff = bi * N
                    gtok = t0 + toff
                    # ---- v for this batch: transpose vT slices -> vx + vpad ----
                    vx = vxp.tile([N, HEADS * (DH + 1)], f32, tag="vx")
                    vxh = vx[:].rearrange("p (h c) -> p h c", c=DH + 1)
                    vp = vpp.tile([N, HEADS * 66], f32r, tag="vp")
                    vph = vp[:].rearrange("p (h c) -> p h c", c=66)
                    nc.sync.dma_start(
                        out=vph[:, :, 0:1],
                        in_=vz_sb[:, 0:16].rearrange("p (h o) -> p h o", o=1))
                    nc.sync.dma_start(
                        out=vph[:, :, 65:66],
                        in_=vz_sb[:, 16:32].rearrange("p (h o) -> p h o", o=1))
                    for a in range(8):
                        pt2 = pa.tile([N, 128], f32, tag="pA")
                        nc.tensor.transpose(
                            pt2[:], vtv[:, a, toff:toff + N], id_sb[:])
                        pv2 = pt2[:].rearrange("p (h c) -> p h c", c=DH)
                        nc.vector.tensor_copy(vxh[:, a * 2:(a + 1) * 2, 0:DH], pv2)
                        nc.vector.tensor_copy(vph[:, a * 2:(a + 1) * 2, 1:65], pv2)
                    nc.vector.memset(vxh[:, :, DH:DH + 1], 1.0)

                    # ---- attention per head (plain f32, tiny mms) ----
                    res = resp.tile([N, DIM], f32, tag="res")
                    for h in range(HEADS):
                        a, poff = h // 2, (h % 2) * 64
                        pd = ps.tile([N, N], f32, tag="pS")
                        nc.tensor.matmul(
                            pd[:],
                            ktv[poff:poff + 64, a, toff:toff + N],
                            qtv[poff:poff + 64, a, toff:toff + N],
                            start=True, stop=True,
                        )
                        ex = expp.tile([N, N], f32, tag="ex")
                        nc.scalar.activation(ex[:], pd[:], Act.Exp, scale=float(DIM) ** -0.5)
                        po = ps.tile([N, N], f32, tag="pS")
                        nc.tensor.matmul(
                            po[:], ex[:], vxh[:, h, :], start=True, stop=True,
                        )
                        rc = recp.tile([N, 1], f32, tag="rc")
                        nc.vector.reciprocal(rc[:], po[:, DH:DH + 1])
                        nc.vector.tensor_scalar_mul(
                            res[:, h * DH:(h + 1) * DH], po[:, 0:DH], rc[:]
                        )

                    # ---- conv 3x3 SAME (f32r, padded image) + BN ----
                    bn = bnp.tile([N, DIM], f32, tag="bn")
                    for half in range(2):
                        y0h, y1h = half * 8, half * 8 + 8
                        pc = pcv.tile([68, 512], f32, tag="pc")
                        pcy = pc[:].rearrange("p (y x) -> p y x", x=64)
                        for t in (4, 0, 1, 2, 3, 5, 6, 7, 8):
                            dy, dx = t // 3 - 1, t % 3 - 1
                            oy0, oy1 = max(y0h, -dy), min(y1h, 16 - dy)
                            nc.tensor.matmul(
                                pcy[:, oy0 - y0h:oy1 - y0h, :],
                                wconv[:, t, :],
                                vph[:, oy0 + dy:oy1 + dy, 1 + dx:1 + dx + 64],
                                start=(t == 4), stop=(t == 8),
                            )
                        nc.vector.tensor_scalar(
                            bn[:, half * 512:half * 512 + 512], pc[0:N, :],
                            st_sb[:, 0:1], st_sb[:, 1:2], Alu.mult, Alu.add,
                        )
                    nc.vector.tensor_add(res[:], res[:], bn[:])

                    # ---- transpose res -> resT [128, 8*68] (cols 65..67 garbage) ----
                    rt = rtp.tile([128, 8 * 68], f32r, tag="rt")
                    for c8 in range(8):
                        pt = pa.tile([128, N], f32, tag="pA")
                        nc.tensor.transpose(
                            pt[:], res[:, c8 * 128:(c8 + 1) * 128], id_sb[:N, :N]
                        )
                        nc.scalar.activation(rt[:, c8 * 68:c8 * 68 + N], pt[:], Act.Copy)

                    # ---- final projection [65, 1024] ----
                    for nt in range(2):
                        f0 = nt * 512
                        pf = p512.tile([68, 512], f32, tag="p512")
                        for ki in range(8):
                            nc.tensor.matmul(
                                pf[:],
                                rt[:, ki * 68:ki * 68 + 68],
                                wout[:, ki, f0:f0 + 512],
                                start=(ki == 0), stop=(ki == 7),
                            )
                        ob = osbp.tile([N, 512], f32, tag="ob")
                        nc.vector.tensor_copy(ob[:], pf[0:N, :])
                        nc.sync.dma_start(
                            out=out[gtok:gtok + N, f0:f0 + 512], in_=ob[:]
                        )
    return nc


def kernel(x, w_qkv, b_qkv, w_out, b_out, conv_w, conv_b,
           bn_gamma, bn_beta, bn_mean, bn_var):
    import concourse.bass as bass
    import concourse.bacc as bacc
    import concourse.mybir as mybir
    from concourse.bass_utils import run_bass_kernel_spmd

    x = np.asarray(x, np.float32)
    xt_all = np.ascontiguousarray(
        x.reshape(B * N, DIM).T)                      # [1024, 16640]
    wqkvt = np.ascontiguousarray(np.asarray(w_qkv, np.float32).T)
    woutt = np.ascontiguousarray(np.asarray(w_out, np.float32).T)
    wc = np.asarray(conv_w, np.float32).transpose(2, 3, 1, 0).reshape(9, N, N)
    wconvt = np.zeros((9, N, 68), np.float32)
    wconvt[:, :, :N] = wc
    s = np.asarray(bn_gamma, np.float32) / np.sqrt(np.asarray(bn_var, np.float32) + BN_EPS)
    t_aff = (np.asarray(conv_b, np.float32) - np.asarray(bn_mean, np.float32)) * s \
        + np.asarray(bn_beta, np.float32)
    st = np.ascontiguousarray(np.stack([s, t_aff], 1))
    bqkvc = np.ascontiguousarray(np.asarray(b_qkv, np.float32).reshape(24, 128).T)
    identm = np.eye(128, dtype=np.float32)

    nc = bacc.Bacc()
    _build(nc, mybir, bass)
    nc.finalize()

    in_maps = []
    for c in range(NCORES):
        in_maps.append({
            "xt": np.ascontiguousarray(xt_all[:, c * TOK:(c + 1) * TOK]),
            "wqkvt": wqkvt, "woutt": woutt, "wconvt": wconvt, "st": st,
            "bqkvc": bqkvc, "ident": identm,
            "vzpad": np.zeros((N, 32), np.float32),
        })
    res = run_bass_kernel_spmd(nc, in_maps, list(range(NCORES)))
    outs = [res.results[c]["out"] for c in range(NCORES)]
    full = np.concatenate(outs, axis=0).reshape(B, N, DIM)

    # exact host-side correction for v/out biases (batch-independent):
    # attn rows sum to 1 -> out1 += b_v; conv(v + b_v_img) = conv(v) + conv(b_v_img)
    b_v = np.asarray(b_qkv, np.float32)[2 * DIM:]
    bimg = b_v.reshape(HEADS, DH)
    pad = np.zeros((HEADS + 2, DH + 2), np.float32)
    pad[1:-1, 1:-1] = bimg
    wsum = np.asarray(conv_w, np.float32).sum(1)      # [65, 3, 3]
    dconv = np.zeros((N, HEADS, DH), np.float32)
    for ty in range(3):
        for tx in range(3):
            dconv += wsum[:, ty, tx][:, None, None] * \
                pad[ty:ty + HEADS, tx:tx + DH][None, :, :]
    dres = b_v[None, :] + (dconv * s[:, None, None]).reshape(N, DIM)
    dout = dres @ woutt + np.asarray(b_out, np.float32)[None, :]
    return full + dout[None, :, :]



# revision 4
# speedup vs baseline: 1.2308x; 1.2308x over previous
import numpy as np

# nn_Attention: B=256, N=65, DIM=1024, HEADS=16, DH=64 across 8 cores (32 batches/core)
B, N, DIM, HEADS, DH = 256, 65, 1024, 16, 64
NCORES = 8
BPC = B // NCORES            # 32 batches per core
TOK = BPC * N                # 2080 tokens per core
BB = 4                       # batches per block
NBLK = BPC // BB             # 8 blocks
TB = BB * N                  # 260 tokens per block
BN_EPS = 1e-5


def _build(nc_mod, mybir, bass):
    f32 = mybir.dt.float32
    bf16 = mybir.dt.bfloat16
    Alu = mybir.AluOpType
    Act = mybir.ActivationFunctionType
    from concourse.tile import TileContext

    nc = nc_mod
    xt = nc.declare_dram_parameter("xt", [DIM, TOK], bf16, isOutput=False)
    wqkvt = nc.declare_dram_parameter("wqkvt", [DIM, 3 * DIM], bf16, isOutput=False)
    woutt = nc.declare_dram_parameter("woutt", [DIM, DIM], bf16, isOutput=False)
    wconvt = nc.declare_dram_parameter("wconvt", [9, N, 68], bf16, isOutput=False)
    st = nc.declare_dram_parameter("st", [N, 2], f32, isOutput=False)
    bqkvc = nc.declare_dram_parameter("bqkvc", [128, 16], f32, isOutput=False)
    ident = nc.declare_dram_parameter("ident", [128, 128], bf16, isOutput=False)
    out = nc.declare_dram_parameter("out", [DIM, TOK], f32, isOutput=True)

    from contextlib import ExitStack
    with TileContext(nc) as tc:
        with ExitStack() as es:
            P = lambda *a, **k: es.enter_context(tc.tile_pool(*a, **k))
            cp = P(name="consts", bufs=1)
            xtp = P(name="xtp", bufs=2)
            qtp = P(name="qtp", bufs=2)
            ktp = P(name="ktp", bufs=2)
            vtp = P(name="vtp", bufs=2)
            vbp = P(name="vbp", bufs=2)
            expp = P(name="exps", bufs=4)
            recp = P(name="recips", bufs=4)
            resp = P(name="resp", bufs=2)
            bnp = P(name="bnp", bufs=2)
            rtp = P(name="rtp", bufs=2)
            osbp = P(name="osb", bufs=2)
            pa = P(name="pa", bufs=2, space="PSUM")
            ps = P(name="ps", bufs=4, space="PSUM")
            pcv = P(name="pconv", bufs=2, space="PSUM")

            # ---- resident constants ----
            wqkv_sb = cp.tile([128, 8 * 3 * DIM], bf16, tag="wqkv")
            nc.sync.dma_start(
                out=wqkv_sb[:].rearrange("p (a n) -> p a n", a=8),
                in_=wqkvt[:].rearrange("(a p) n -> p a n", p=128),
            )
            wqkv = wqkv_sb[:].rearrange("p (a n) -> p a n", a=8)

            wout_sb = cp.tile([128, 8 * DIM], bf16, tag="wout")
            nc.sync.dma_start(
                out=wout_sb[:].rearrange("p (a n) -> p a n", a=8),
                in_=woutt[:].rearrange("(a p) n -> p a n", p=128),
            )
            wout = wout_sb[:].rearrange("p (a n) -> p a n", a=8)

            wconv_sb = cp.tile([N, 9 * 68], bf16, tag="wconv")
            nc.sync.dma_start(
                out=wconv_sb[:].rearrange("i (t o) -> i t o", t=9),
                in_=wconvt[:].rearrange("t i o -> i t o"),
            )
            wconv = wconv_sb[:].rearrange("i (t o) -> i t o", t=9)

            st_sb = cp.tile([N, 2], f32, tag="st")
            nc.sync.dma_start(out=st_sb[:], in_=st[:])
            bq_sb = cp.tile([128, 16], f32, tag="bq")
            nc.sync.dma_start(out=bq_sb[:], in_=bqkvc[:])
            id_sb = cp.tile([128, 128], bf16, tag="id")
            nc.sync.dma_start(out=id_sb[:], in_=ident[:])

            for blk in range(NBLK):
                t0 = blk * TB
                xt_sb = xtp.tile([128, 8 * TB], bf16, tag="xt")
                xtv = xt_sb[:].rearrange("p (a n) -> p a n", a=8)
                nc.sync.dma_start(
                    out=xtv,
                    in_=xt[:].rearrange("(a p) n -> p a n", p=128)[:, :, t0:t0 + TB],
                )

                # ---- Q^T, K^T, V^T projections: [feat 128-tile, TB] bf16 ----
                qt_sb = qtp.tile([128, 8 * TB], bf16, tag="qt")
                qtv = qt_sb[:].rearrange("p (a n) -> p a n", a=8)
                kt_sb = ktp.tile([128, 8 * TB], bf16, tag="kt")
                ktv = kt_sb[:].rearrange("p (a n) -> p a n", a=8)
                vt_sb = vtp.tile([128, 8 * TB], bf16, tag="vt")
                vtv = vt_sb[:].rearrange("p (a n) -> p a n", a=8)
                for dst, coff, boff in ((qtv, 0, 0), (ktv, DIM, 8), (vtv, 2 * DIM, None)):
                    for m in range(8):
                        pqk = pa.tile([128, TB], f32, tag="pA")
                        for ki in range(8):
                            nc.tensor.matmul(
                                pqk[:],
                                wqkv[:, ki, coff + m * 128:coff + (m + 1) * 128],
                                xtv[:, ki, :],
                                start=(ki == 0), stop=(ki == 7),
                            )
                        if boff is not None:
                            nc.vector.tensor_scalar_add(
                                dst[:, m, :], pqk[:], bq_sb[:, boff + m:boff + m + 1]
                            )
                        else:
                            nc.scalar.activation(dst[:, m, :], pqk[:], Act.Copy)

                rt = rtp.tile([128, 8 * TB], bf16, tag="rt")
                rtv = rt[:].rearrange("p (a n) -> p a n", a=8)

                for bi in range(BB):
                    toff = bi * N
                    # ---- per-batch v tile [65, 16*(64+1)]; col 64 of head = ones ----
                    vb = vbp.tile([N, HEADS * (DH + 1)], bf16, tag="vb")
                    vbh = vb[:].rearrange("p (h c) -> p h c", c=DH + 1)
                    nc.vector.memset(vbh[:, :, DH:DH + 1], 1.0)
                    for a in range(8):
                        pt2 = ps.tile([N, 128], bf16, tag="pS")
                        nc.tensor.transpose(
                            pt2[:], vtv[:, a, toff:toff + N], id_sb[:])
                        nc.scalar.activation(
                            vbh[:, a * 2:(a + 1) * 2, 0:DH],
                            pt2[:].rearrange("p (h c) -> p h c", c=DH), Act.Copy)

                    # ---- attention per head ----
                    res = resp.tile([N, DIM], bf16, tag="res")
                    for h in range(HEADS):
                        a, poff = h // 2, (h % 2) * 64
                        pd = ps.tile([N, N], f32, tag="pS")
                        nc.tensor.matmul(
                            pd[:],
                            ktv[poff:poff + 64, a, toff:toff + N],
                            qtv[poff:poff + 64, a, toff:toff + N],
                            start=True, stop=True,
                        )
                        ex = expp.tile([N, N], bf16, tag="ex")
                        nc.scalar.activation(ex[:], pd[:], Act.Exp, scale=float(DIM) ** -0.5)
                        po = ps.tile([N, DH + 1], f32, tag="pS")
                        nc.tensor.matmul(
                            po[:], ex[:], vbh[:, h, :], start=True, stop=True,
                        )
                        rc = recp.tile([N, 1], f32, tag="rc")
                        nc.vector.reciprocal(rc[:], po[:, DH:DH + 1])
                        nc.vector.tensor_scalar_mul(
                            res[:, h * DH:(h + 1) * DH], po[:, 0:DH], rc[:]
                        )

                    # ---- conv 3x3 SAME (sliced taps, no padding) + BN ----
                    bn = bnp.tile([N, DIM], bf16, tag="bn")
                    for half in range(2):
                        y0h, y1h = half * 8, half * 8 + 8
                        pc = pcv.tile([68, 512], f32, tag="pc")
                        pcy = pc[:].rearrange("p (y x) -> p y x", x=64)
                        for t in (4, 0, 1, 2, 3, 5, 6, 7, 8):
                            dy, dx = t // 3 - 1, t % 3 - 1
                            oy0, oy1 = max(y0h, -dy), min(y1h, 16 - dy)
                            ox0, ox1 = max(0, -dx), min(DH, DH - dx)
                            nc.tensor.matmul(
                                pcy[:, oy0 - y0h:oy1 - y0h, ox0:ox1],
                                wconv[:, t, :],
                                vbh[:, oy0 + dy:oy1 + dy, ox0 + dx:ox1 + dx],
                                start=(t == 4), stop=(t == 8),
                            )
                        nc.vector.tensor_scalar(
                            bn[:, half * 512:half * 512 + 512], pc[0:N, :],
                            st_sb[:, 0:1], st_sb[:, 1:2], Alu.mult, Alu.add,
                        )
                    nc.vector.tensor_add(res[:], res[:], bn[:])

                    # ---- transpose res into rt [128, 8ki, 260] bf16 ----
                    for c8 in range(8):
                        pt = ps.tile([128, N], bf16, tag="pS")
                        nc.tensor.transpose(
                            pt[:], res[:, c8 * 128:(c8 + 1) * 128], id_sb[:N, :N]
                        )
                        nc.scalar.activation(rtv[:, c8, toff:toff + N], pt[:], Act.Copy)

                # ---- final projection, transposed out: [1024, 260] per block ----
                for m in range(8):
                    po2 = pa.tile([128, TB], f32, tag="pA")
                    for ki in range(8):
                        nc.tensor.matmul(
                            po2[:],
                            wout[:, ki, m * 128:(m + 1) * 128],
                            rtv[:, ki, :],
                            start=(ki == 0), stop=(ki == 7),
                        )
                    ob = osbp.tile([128, TB], f32, tag="ob")
                    nc.vector.tensor_copy(ob[:], po2[:])
                    nc.sync.dma_start(
                        out=out[m * 128:(m + 1) * 128, t0:t0 + TB], in_=ob[:]
                    )
    return nc


def kernel(x, w_qkv, b_qkv, w_out, b_out, conv_w, conv_b,
           bn_gamma, bn_beta, bn_mean, bn_var):
    import concourse.bass as bass
    import concourse.bacc as bacc
    import concourse.mybir as mybir
    from concourse.bass_utils import run_bass_kernel_spmd
    import ml_dtypes
    bf = ml_dtypes.bfloat16

    x = np.asarray(x, np.float32)
    xt_all = np.ascontiguousarray(
        x.reshape(B * N, DIM).T.astype(bf))                  # [1024, 16640] bf16
    wqkvt = np.ascontiguousarray(np.asarray(w_qkv, np.float32).T.astype(bf))
    woutt_f32 = np.ascontiguousarray(np.asarray(w_out, np.float32).T)
    woutt = np.ascontiguousarray(woutt_f32.astype(bf))
    wc = np.asarray(conv_w, np.float32).transpose(2, 3, 1, 0).reshape(9, N, N)
    wconvt = np.zeros((9, N, 68), bf)
    wconvt[:, :, :N] = wc.astype(bf)
    s = np.asarray(bn_gamma, np.float32) / np.sqrt(np.asarray(bn_var, np.float32) + BN_EPS)
    t_aff = (np.asarray(conv_b, np.float32) - np.asarray(bn_mean, np.float32)) * s \
        + np.asarray(bn_beta, np.float32)
    st = np.ascontiguousarray(np.stack([s, t_aff], 1))
    bqkvc = np.ascontiguousarray(
        np.asarray(b_qkv, np.float32)[:2 * DIM].reshape(16, 128).T)
    identm = np.eye(128, dtype=bf)

    nc = bacc.Bacc()
    _build(nc, mybir, bass)
    nc.finalize()

    in_maps = []
    for c in range(NCORES):
        in_maps.append({
            "xt": np.ascontiguousarray(xt_all[:, c * TOK:(c + 1) * TOK]),
            "wqkvt": wqkvt, "woutt": woutt, "wconvt": wconvt, "st": st,
            "bqkvc": bqkvc, "ident": identm,
        })
    res = run_bass_kernel_spmd(nc, in_maps, list(range(NCORES)))
    outs = [res.results[c]["out"] for c in range(NCORES)]   # each [1024, 2080]
    full = np.concatenate(outs, axis=1).T.reshape(B, N, DIM)

    # exact host-side correction for v/out biases (batch-independent):
    # attn rows sum to 1 -> out1 += b_v; conv(v + b_v_img) = conv(v) + conv(b_v_img)
    b_v = np.asarray(b_qkv, np.float32)[2 * DIM:]
    bimg = b_v.reshape(HEADS, DH)
    pad = np.zeros((HEADS + 2, DH + 2), np.float32)
    pad[1:-1, 1:-1] = bimg
    wsum = np.asarray(conv_w, np.float32).sum(1)      # [65, 3, 3]
    dconv = np.zeros((N, HEADS, DH), np.float32)
    for ty in range(3):
        for tx in range(3):
            dconv += wsum[:, ty, tx][:, None, None] * \
                pad[ty:ty + HEADS, tx:tx + DH][None, :, :]
    dres = b_v[None, :] + (dconv * s[:, None, None]).reshape(N, DIM)
    dout = dres @ woutt_f32 + np.asarray(b_out, np.float32)[None, :]
    return full + dout[None, :, :]


# revision 11
# speedup vs baseline: 1.6081x; 1.3066x over previous
import numpy as np

# nn_Attention: B=256, N=65, DIM=1024, HEADS=16, DH=64 across 8 cores (32 batches/core)
B, N, DIM, HEADS, DH = 256, 65, 1024, 16, 64
NCORES = 8
BPC = B // NCORES            # 32 batches per core
TOK = BPC * N                # 2080 tokens per core
BB = 4                       # batches per block
NBLK = BPC // BB             # 8 blocks
TB = BB * N                  # 260 tokens per block
BN_EPS = 1e-5


def _build(nc_mod, mybir, bass):
    f32 = mybir.dt.float32
    bf16 = mybir.dt.bfloat16
    Alu = mybir.AluOpType
    Act = mybir.ActivationFunctionType
    from concourse.tile import TileContext

    nc = nc_mod
    xt = nc.declare_dram_parameter("xt", [DIM, TOK], bf16, isOutput=False)
    wqkvt = nc.declare_dram_parameter("wqkvt", [DIM, 3 * DIM], bf16, isOutput=False)
    woutt = nc.declare_dram_parameter("woutt", [DIM, DIM], bf16, isOutput=False)
    wconvt = nc.declare_dram_parameter("wconvt", [9, N, 68], bf16, isOutput=False)
    st = nc.declare_dram_parameter("st", [N, 2], f32, isOutput=False)
    bqkvc = nc.declare_dram_parameter("bqkvc", [128, 16], f32, isOutput=False)
    ident = nc.declare_dram_parameter("ident", [128, 128], bf16, isOutput=False)
    out = nc.declare_dram_parameter("out", [DIM, TOK], f32, isOutput=True)

    from contextlib import ExitStack
    with TileContext(nc) as tc:
        with ExitStack() as es:
            P = lambda *a, **k: es.enter_context(tc.tile_pool(*a, **k))
            cp = P(name="consts", bufs=1)
            xtp = P(name="xtp", bufs=2)
            qtp = P(name="qtp", bufs=2)
            ktp = P(name="ktp", bufs=2)
            vtp = P(name="vtp", bufs=2)
            vbp = P(name="vbp", bufs=2)
            expp = P(name="exps", bufs=4)
            recp = P(name="recips", bufs=4)
            resp = P(name="resp", bufs=2)
            bnp = P(name="bnp", bufs=2)
            rtp = P(name="rtp", bufs=2)
            osbp = P(name="osb", bufs=2)
            pa = P(name="pa", bufs=2, space="PSUM")     # [128,260] f32: QKV evac + outproj
            psml = P(name="psml", bufs=2, space="PSUM")  # [65/128,<=260]: dots4 + transposes
            pat = P(name="pat", bufs=2, space="PSUM")   # [65,260] f32: attnV 4-head groups
            pcv = P(name="pconv", bufs=2, space="PSUM")  # [68,512] f32: conv halves

            # ---- resident constants ----
            wqkv_sb = cp.tile([128, 8 * 3 * DIM], bf16, tag="wqkv")
            nc.sync.dma_start(
                out=wqkv_sb[:].rearrange("p (a n) -> p a n", a=8),
                in_=wqkvt[:].rearrange("(a p) n -> p a n", p=128),
            )
            wqkv = wqkv_sb[:].rearrange("p (a n) -> p a n", a=8)

            wout_sb = cp.tile([128, 8 * DIM], bf16, tag="wout")
            nc.sync.dma_start(
                out=wout_sb[:].rearrange("p (a n) -> p a n", a=8),
                in_=woutt[:].rearrange("(a p) n -> p a n", p=128),
            )
            wout = wout_sb[:].rearrange("p (a n) -> p a n", a=8)

            wconv_sb = cp.tile([N, 9 * 68], bf16, tag="wconv")
            nc.sync.dma_start(
                out=wconv_sb[:].rearrange("i (t o) -> i t o", t=9),
                in_=wconvt[:].rearrange("t i o -> i t o"),
            )
            wconv = wconv_sb[:].rearrange("i (t o) -> i t o", t=9)

            st_sb = cp.tile([N, 2], f32, tag="st")
            nc.sync.dma_start(out=st_sb[:], in_=st[:])
            bq_sb = cp.tile([128, 16], f32, tag="bq")
            nc.sync.dma_start(out=bq_sb[:], in_=bqkvc[:])
            id_sb = cp.tile([128, 128], bf16, tag="id")
            nc.sync.dma_start(out=id_sb[:], in_=ident[:])

            SCALE = float(DIM) ** -0.5

            for blk in range(NBLK):
                t0 = blk * TB
                xt_sb = xtp.tile([128, 8 * TB], bf16, tag="xt")
                xtv = xt_sb[:].rearrange("p (a n) -> p a n", a=8)
                nc.sync.dma_start(
                    out=xtv,
                    in_=xt[:].rearrange("(a p) n -> p a n", p=128)[:, :, t0:t0 + TB],
                )

                # ---- Q^T, K^T, V^T projections: [feat 128-tile, TB] bf16 ----
                qt_sb = qtp.tile([128, 8 * TB], bf16, tag="qt")
                qtv = qt_sb[:].rearrange("p (a n) -> p a n", a=8)
                kt_sb = ktp.tile([128, 8 * TB], bf16, tag="kt")
                ktv = kt_sb[:].rearrange("p (a n) -> p a n", a=8)
                vt_sb = vtp.tile([128, 8 * TB], bf16, tag="vt")
                vtv = vt_sb[:].rearrange("p (a n) -> p a n", a=8)
                for dst, coff, boff in ((qtv, 0, 0), (ktv, DIM, 8), (vtv, 2 * DIM, None)):
                    for m in range(8):
                        pqk = pa.tile([128, TB], f32, tag="pA")
                        for ki in range(8):
                            nc.tensor.matmul(
                                pqk[:],
                                wqkv[:, ki, coff + m * 128:coff + (m + 1) * 128],
                                xtv[:, ki, :],
                                start=(ki == 0), stop=(ki == 7),
                            )
                        if boff is not None:
                            nc.vector.tensor_scalar_add(
                                dst[:, m, :], pqk[:], bq_sb[:, boff + m:boff + m + 1]
                            )
                        else:
                            nc.scalar.activation(dst[:, m, :], pqk[:], Act.Copy)

                rt = rtp.tile([128, 8 * TB], bf16, tag="rt")
                rtv = rt[:].rearrange("p (a n) -> p a n", a=8)

                for bi in range(BB):
                    toff = bi * N
                    # ---- per-batch v tile [65, 1+16*66(+1 slack)] ----
                    # col 0 zero; head h at 1+66h: v(64), ones, zero.
                    # attention reads [v|ones]; conv dx=-1 reads the
                    # preceding zero col; dx=+1 spuriously reads the ones
                    # col into out x=63 (corrected host-side).
                    vb = vbp.tile([N, 1 + HEADS * 66 + 1], bf16, tag="vb")
                    vbh = vb[:, 1:1 + HEADS * 66].rearrange("p (h c) -> p h c", c=66)
                    nc.gpsimd.memset(
                        vb[:, 0:HEADS * 66].rearrange(
                            "p (h c) -> p h c", c=66)[:, :, 0:1], 0.0)
                    nc.gpsimd.memset(vbh[:, :, DH:DH + 1], 1.0)
                    nc.gpsimd.memset(vbh[:, HEADS - 1:HEADS, DH + 1:DH + 2], 0.0)
                    for a in range(8):
                        pt2 = psml.tile([N, 260], bf16, tag="pS")
                        nc.tensor.transpose(
                            pt2[:, 0:128], vtv[:, a, toff:toff + N], id_sb[:])
                        if a % 2 == 0:
                            nc.vector.tensor_copy(
                                vbh[:, a * 2:(a + 1) * 2, 0:DH],
                                pt2[:, 0:128].rearrange("p (h c) -> p h c", c=DH))
                        else:
                            nc.scalar.activation(
                                vbh[:, a * 2:(a + 1) * 2, 0:DH],
                                pt2[:, 0:128].rearrange("p (h c) -> p h c", c=DH),
                                Act.Copy)

                    res = resp.tile([N, DIM], bf16, tag="res")
                    bn = bnp.tile([N, DIM], bf16, tag="bn")

                    # attention in 4-head groups, conv halves interleaved to
                    # keep the PE busy while scalar exp runs.
                    # heads grouped by parity: alternating the stationary
                    # base-partition (0/64) between back-to-back matmuls into
                    # one psum tile faults on HW, so keep poff constant
                    # within each group.
                    HG = ([0, 2, 4, 6], [1, 3, 5, 7],
                          [8, 10, 12, 14], [9, 11, 13, 15])
                    pd_tiles = {}
                    ex_tiles = {}

                    def dots4(g):
                        pd4 = psml.tile([N, 260], f32, tag="pS")
                        for j, h in enumerate(HG[g]):
                            a, poff = h // 2, (h % 2) * 64
                            nc.tensor.matmul(
                                pd4[:, j * N:(j + 1) * N],
                                ktv[poff:poff + 64, a, toff:toff + N],
                                qtv[poff:poff + 64, a, toff:toff + N],
                                start=True, stop=True,
                            )
                        pd_tiles[g] = pd4

                    def exp4(g):
                        ex4 = expp.tile([N, 260], bf16, tag="ex")
                        nc.scalar.activation(ex4[:], pd_tiles[g][:], Act.Exp, scale=SCALE)
                        ex_tiles[g] = ex4

                    def attn4(g):
                        po4 = pat.tile([N, 260], f32, tag="po")
                        ex4 = ex_tiles[g]
                        for j, h in enumerate(HG[g]):
                            nc.tensor.matmul(
                                po4[:, j * N:(j + 1) * N],
                                ex4[:, j * N:(j + 1) * N],
                                vbh[:, h, 0:DH + 1],
                                start=True, stop=True,
                            )
                        pov = po4[:].rearrange("p (j c) -> p j c", c=N)
                        rc4 = recp.tile([N, 4], f32, tag="rc")
                        nc.vector.reciprocal(rc4[:], pov[:, :, DH])
                        par, jb = g % 2, (g // 2) * 4
                        nc.vector.tensor_tensor(
                            res[:].rearrange("p (j q c) -> p j q c", q=2, c=DH)[
                                :, jb:jb + 4, par:par + 1, :],
                            po4[:].rearrange("p (j o c) -> p j o c", o=1, c=N)[
                                :, :, :, 0:DH],
                            rc4[:].rearrange("p (j o q) -> p j o q", o=1, q=1
                                             ).to_broadcast([N, 4, 1, DH]),
                            Alu.mult,
                        )

                    def conv_half(half):
                        y0h, y1h = half * 8, half * 8 + 8
                        pc = pcv.tile([68, 512], f32, tag="pc")
                        pcy = pc[:].rearrange("p (y x) -> p y x", x=64)
                        for t in (4, 0, 1, 2, 3, 5, 6, 7, 8):
                            dy, dx = t // 3 - 1, t % 3 - 1
                            oy0, oy1 = max(y0h, -dy), min(y1h, 16 - dy)
                            ny = oy1 - oy0
                            off = 1 + dx + 66 * (oy0 + dy)
                            mov = vb[:, off:off + 66 * ny].rearrange(
                                "p (y c) -> p y c", c=66)[:, :, 0:DH]
                            nc.tensor.matmul(
                                pcy[:, oy0 - y0h:oy1 - y0h, :],
                                wconv[:, t, :],
                                mov,
                                start=(t == 4), stop=(t == 8),
                            )
                        nc.vector.tensor_scalar(
                            bn[:, half * 512:half * 512 + 512], pc[0:N, :],
                            st_sb[:, 0:1], 0.0, Alu.mult, Alu.add,
                        )

                    dots4(0)
                    dots4(1)
                    exp4(0)
                    exp4(1)
                    conv_half(0)
                    attn4(0)
                    attn4(1)
                    dots4(2)
                    dots4(3)
                    exp4(2)
                    exp4(3)
                    conv_half(1)
                    attn4(2)
                    attn4(3)

                    nc.vector.tensor_add(res[:], res[:], bn[:])

                    # ---- transpose res into rt [128, 8ki, 260] bf16 ----
                    for c8 in range(8):
                        pt = psml.tile([128, 260], bf16, tag="pS")
                        nc.tensor.transpose(
                            pt[:, 0:N], res[:, c8 * 128:(c8 + 1) * 128], id_sb[:N, :N]
                        )
                        if c8 % 2 == 0:
                            nc.vector.tensor_copy(rtv[:, c8, toff:toff + N], pt[:, 0:N])
                        else:
                            nc.scalar.activation(rtv[:, c8, toff:toff + N], pt[:, 0:N],
                                                 Act.Copy)

                # ---- final projection, transposed out: [1024, 260] per block ----
                for m in range(8):
                    po2 = pa.tile([128, TB], f32, tag="pA")
                    for ki in range(8):
                        nc.tensor.matmul(
                            po2[:],
                            wout[:, ki, m * 128:(m + 1) * 128],
                            rtv[:, ki, :],
                            start=(ki == 0), stop=(ki == 7),
                        )
                    ob = osbp.tile([128, TB], f32, tag="ob")
                    nc.vector.tensor_copy(ob[:], po2[:])
                    nc.sync.dma_start(
                        out=out[m * 128:(m + 1) * 128, t0:t0 + TB], in_=ob[:]
                    )
    return nc


def kernel(x, w_qkv, b_qkv, w_out, b_out, conv_w, conv_b,
           bn_gamma, bn_beta, bn_mean, bn_var):
    import concourse.bass as bass
    import concourse.bacc as bacc
    import concourse.mybir as mybir
    from concourse.bass_utils import run_bass_kernel_spmd
    import ml_dtypes
    bf = ml_dtypes.bfloat16

    x = np.asarray(x, np.float32)
    xt_all = np.ascontiguousarray(
        x.reshape(B * N, DIM).T.astype(bf))                  # [1024, 16640] bf16
    wqkvt = np.ascontiguousarray(np.asarray(w_qkv, np.float32).T.astype(bf))
    woutt_f32 = np.ascontiguousarray(np.asarray(w_out, np.float32).T)
    woutt = np.ascontiguousarray(woutt_f32.astype(bf))
    wc = np.asarray(conv_w, np.float32).transpose(2, 3, 1, 0).reshape(9, N, N)
    wconvt = np.zeros((9, N, 68), bf)
    wconvt[:, :, :N] = wc.astype(bf)
    s = np.asarray(bn_gamma, np.float32) / np.sqrt(np.asarray(bn_var, np.float32) + BN_EPS)
    t_aff = (np.asarray(conv_b, np.float32) - np.asarray(bn_mean, np.float32)) * s \
        + np.asarray(bn_beta, np.float32)
    st = np.ascontiguousarray(np.stack([s, np.zeros_like(s)], 1))
    bqkvc = np.ascontiguousarray(
        np.asarray(b_qkv, np.float32)[:2 * DIM].reshape(16, 128).T)
    identm = np.eye(128, dtype=bf)

    nc = bacc.Bacc()
    _build(nc, mybir, bass)
    nc.finalize()

    in_maps = []
    for c in range(NCORES):
        in_maps.append({
            "xt": np.ascontiguousarray(xt_all[:, c * TOK:(c + 1) * TOK]),
            "wqkvt": wqkvt, "woutt": woutt, "wconvt": wconvt, "st": st,
            "bqkvc": bqkvc, "ident": identm,
        })
    res = run_bass_kernel_spmd(nc, in_maps, list(range(NCORES)))
    outs = [res.results[c]["out"] for c in range(NCORES)]   # each [1024, 2080]
    full = np.concatenate(outs, axis=1).T.reshape(B, N, DIM)

    # exact host-side correction, batch-independent:
    #  - v/out biases: attn rows sum to 1 -> out1 += b_v; conv bias-image effect
    #  - BN additive term t_aff[n] (dropped in-kernel) contributes t_aff[n]
    #    at every feature of token n
    b_v = np.asarray(b_qkv, np.float32)[2 * DIM:]
    bimg = b_v.reshape(HEADS, DH)
    pad = np.zeros((HEADS + 2, DH + 2), np.float32)
    pad[1:-1, 1:-1] = bimg
    wsum = np.asarray(conv_w, np.float32).sum(1)      # [65, 3, 3]
    dconv = np.zeros((N, HEADS, DH), np.float32)
    for ty in range(3):
        for tx in range(3):
            dconv += wsum[:, ty, tx][:, None, None] * \
                pad[ty:ty + HEADS, tx:tx + DH][None, :, :]
    dres = b_v[None, :] + (dconv * s[:, None, None]).reshape(N, DIM) \
        + t_aff[:, None]
    # subtract the spurious dx=+1 ones-column contamination at x=63:
    # tap (ty, tx=2) adds wsum[n, ty] to conv[n, y, 63] for its valid y range
    corr = np.zeros((N, HEADS), np.float32)
    corr += wsum[:, 1, 2][:, None]                      # dy=0: all y
    corr[:, 1:] += wsum[:, 0, 2][:, None]               # dy=-1: y>=1
    corr[:, :HEADS - 1] += wsum[:, 2, 2][:, None]       # dy=+1: y<=14
    for y in range(HEADS):
        dres[:, y * DH + DH - 1] -= corr[:, y] * s
    dout = dres @ woutt_f32 + np.asarray(b_out, np.float32)[None, :]
    return full + dout[None, :, :]
